# revision 5
# baseline (speedup 1.0000x reference)
"""Trainium2 Bass kernel for BatchShawMultigraphAttention.

Math (derived from the reference):
  - attn_biases adds a per-row constant to scores -> cancels in softmax.
  - w.sum(-1) == 1 after softmax, so the bias term reduces to "+ biases[e,h]".
  - masked softmax with -1e10 == multiply exp(scores) by binary A (rows are
    never fully masked at 10% density, N=1024).
  So per (b,e,h):
    P = exp(q @ k^T / sqrt(F));  T = A * P
    out = relu( (T @ (v + bias_eh)) / (T @ 1) )

Sharding: 8 cores = (b in 0..3) x (query-row half in 0..1); each core owns
512 softmax rows for all (e,h), reading its A slice exactly once.
Device layout trick: everything is computed transposed ([j, i]) so A can be
host-pre-transposed and streamed contiguously, and the final matmul
T^T-slices (lhsT) produce the output in natural [i, feat] layout directly.
"""

import sys

sys.path.insert(0, "/opt/trn_rl_repo")

import numpy as np
import ml_dtypes

B, E, H, N, F, F_ = 4, 4, 4, 1024, 64, 32
NCORES = 8
IH = N // 2          # 512 query rows per core
JB = N // 128        # 8 key blocks
VA_W = F_ + 1        # v columns + ones column = 33

_compiled = None


def _build():
    import concourse.bass as bass
    import concourse.bacc as bacc
    import concourse.tile as tile
    import concourse.mybir as mybir

    f32 = mybir.dt.float32
    nc = bacc.Bacc("TRN2", target_bir_lowering=False, debug=False,
                   enable_asserts=False, num_devices=NCORES)

    qt_d = nc.dram_tensor("qt", [H, F_, IH], f32, kind="ExternalInput")
    kt_d = nc.dram_tensor("kt", [H, F_, N], f32, kind="ExternalInput")
    va_d = nc.dram_tensor("va", [E, H, JB, 128, VA_W], f32, kind="ExternalInput")
    at_d = nc.dram_tensor("at", [E, JB, 128, IH], f32, kind="ExternalInput")
    out_d = nc.dram_tensor("out", [IH, E * H * F_], f32, kind="ExternalOutput")

    with tile.TileContext(nc) as tc:
        with (
            tc.tile_pool(name="const", bufs=1) as cpool,
            tc.tile_pool(name="pt", bufs=1) as ptpool,
            tc.tile_pool(name="at", bufs=3) as atpool,
            tc.tile_pool(name="tt", bufs=1) as ttpool,
            tc.tile_pool(name="ps", bufs=2, space=bass.MemorySpace.PSUM) as pspool,
            tc.tile_pool(name="po", bufs=6, space=bass.MemorySpace.PSUM) as popool,
            tc.tile_pool(name="eps", bufs=8) as epool,
        ):
            kt_t, qt_t = [], []
            for h in range(H):
                kt = cpool.tile([F_, N], f32, tag=f"kt{h}")
                nc.sync.dma_start(kt[:], kt_d[h])
                kt_t.append(kt)
                qt = cpool.tile([F_, IH], f32, tag=f"qt{h}")
                nc.sync.dma_start(qt[:], qt_d[h])
                qt_t.append(qt)
            va_t = cpool.tile([128, E * H * JB * VA_W], f32, tag="va")
            for e in range(E):
                for h in range(H):
                    for jb in range(JB):
                        s = ((e * H + h) * JB + jb) * VA_W
                        nc.sync.dma_start(va_t[:, s:s + VA_W], va_d[e, h, jb])

            # Phase B: P^T[h, jb] = exp(k_jb @ q^T / 8)   [128 j, 512 i]
            pt_t = {}
            for h in range(H):
                for jb in range(JB):
                    st = pspool.tile([128, IH], f32, tag="st")
                    nc.tensor.matmul(st[:], kt_t[h][:, jb * 128:(jb + 1) * 128],
                                     qt_t[h][:], start=True, stop=True)
                    pt = ptpool.tile([128, IH], f32, tag=f"pt{h}_{jb}")
                    nc.scalar.activation(pt[:], st[:],
                                         mybir.ActivationFunctionType.Exp,
                                         scale=float(1.0 / np.sqrt(F_)))
                    pt_t[h, jb] = pt

            # Phase C: per e: T^T = A^T * P^T, then out = T @ va (accum over jb)
            for e in range(E):
                tt_t = {}
                for jb in range(JB):
                    at = atpool.tile([128, IH], f32, tag="at")
                    nc.sync.dma_start(at[:], at_d[e, jb])
                    for h in range(H):
                        tt = ttpool.tile([128, IH], f32, tag=f"tt{h}_{jb}")
                        nc.vector.tensor_mul(tt[:], at[:], pt_t[h, jb][:])
                        tt_t[h, jb] = tt
                for h in range(H):
                    col = (e * H + h) * F_
                    for ib in range(IH // 128):
                        po = popool.tile([128, VA_W], f32, tag="po")
                        for jb in range(JB):
                            s = ((e * H + h) * JB + jb) * VA_W
                            nc.tensor.matmul(
                                po[:], tt_t[h, jb][:, ib * 128:(ib + 1) * 128],
                                va_t[:, s:s + VA_W],
                                start=(jb == 0), stop=(jb == JB - 1))
                        rec = epool.tile([128, 1], f32, tag="rec")
                        nc.vector.reciprocal(rec[:], po[:, F_:F_ + 1])
                        ot = epool.tile([128, F_], f32, tag="ot")
                        nc.scalar.activation(ot[:], po[:, 0:F_],
                                             mybir.ActivationFunctionType.Relu,
                                             scale=rec[:])
                        nc.sync.dma_start(
                            out_d[ib * 128:(ib + 1) * 128, col:col + F_], ot[:])

    nc.compile()
    return nc


def _prep_core_inputs(b, ih, X, A, kernel_w, biases, aks, akn):
    i0 = ih * IH
    Xb = X[b]                                        # [N, F]
    qt = np.einsum("nf,hfk->hkn", Xb[i0:i0 + IH], aks).astype(np.float32)
    kt = np.einsum("nf,hfk->hkn", Xb, akn).astype(np.float32)
    v = np.einsum("nf,hfk->hnk", Xb, kernel_w)       # [H, N, F_]
    va = np.empty((E, H, JB, 128, VA_W), np.float32)
    for e in range(E):
        for h in range(H):
            vb = v[h] + biases[e, h][None, :]        # [N, F_]
            va[e, h, :, :, :F_] = vb.reshape(JB, 128, F_)
            va[e, h, :, :, F_] = 1.0
    at = np.ascontiguousarray(
        A[b, :, i0:i0 + IH, :].transpose(0, 2, 1)    # [E, N_j, IH]
    ).reshape(E, JB, 128, IH).astype(np.float32)
    return {"qt": qt, "kt": kt, "va": va, "at": at}


def kernel(X, A, kernel, biases, attn_kernel_self, attn_kernel_neighs,
           attn_biases):
    global _compiled
    from concourse import bass_utils

    if _compiled is None:
        _compiled = _build()

    in_maps = [
        _prep_core_inputs(c // 2, c % 2, np.asarray(X), np.asarray(A),
                          np.asarray(kernel), np.asarray(biases),
                          np.asarray(attn_kernel_self),
                          np.asarray(attn_kernel_neighs))
        for c in range(NCORES)
    ]
    res = bass_utils.run_bass_kernel_spmd(_compiled, in_maps,
                                          core_ids=list(range(NCORES)))
    out = np.empty((B, N, E * H * F_), np.float32)
    for c in range(NCORES):
        b, ih = c // 2, c % 2
        out[b, ih * IH:(ih + 1) * IH, :] = res.results[c]["out"]
    return out


# revision 7
# speedup vs baseline: 1.1223x; 1.1223x over previous
"""Trainium2 Bass kernel for BatchShawMultigraphAttention.

Math (derived from the reference):
  - attn_biases adds a per-row constant to scores -> cancels in softmax.
  - w.sum(-1) == 1 after softmax, so the bias term reduces to "+ biases[e,h]".
  - masked softmax with -1e10 == multiply exp(scores) by binary A (rows are
    never fully masked at 10% density, N=1024).
  So per (b,e,h):
    P = exp(q @ k^T / sqrt(F));  T = A * P
    out = relu( (T @ (v + bias_eh)) / (T @ 1) )

Sharding: 8 cores = (b in 0..3) x (query-row half in 0..1); each core owns
512 softmax rows for all (e,h), reading its A slice exactly once.
Device layout trick: everything is computed transposed ([j, i]) so A can be
host-pre-transposed and streamed contiguously, and the final matmul
T^T-slices (lhsT) produce the output in natural [i, feat] layout directly.
"""

import sys

sys.path.insert(0, "/opt/trn_rl_repo")

import numpy as np
import ml_dtypes

B, E, H, N, F, F_ = 4, 4, 4, 1024, 64, 32
NCORES = 8
IH = N // 2          # 512 query rows per core
JB = N // 128        # 8 key blocks
VA_W = F_ + 1        # v columns + ones column = 33

_compiled = None


def _build():
    import concourse.bass as bass
    import concourse.bacc as bacc
    import concourse.tile as tile
    import concourse.mybir as mybir

    f32 = mybir.dt.float32
    nc = bacc.Bacc("TRN2", target_bir_lowering=False, debug=False,
                   enable_asserts=False, num_devices=NCORES)

    qt_d = nc.dram_tensor("qt", [H, F_, IH], f32, kind="ExternalInput")
    kt_d = nc.dram_tensor("kt", [H, F_, N], f32, kind="ExternalInput")
    va_d = nc.dram_tensor("va", [E, H, JB, 128, VA_W], f32, kind="ExternalInput")
    at_d = nc.dram_tensor("at", [E, JB, 128, IH], f32, kind="ExternalInput")
    out_d = nc.dram_tensor("out", [IH, E * H * F_], f32, kind="ExternalOutput")

    with tile.TileContext(nc) as tc:
        with (
            tc.tile_pool(name="const", bufs=1) as cpool,
            tc.tile_pool(name="pt", bufs=1) as ptpool,
            tc.tile_pool(name="at", bufs=1) as atpool,
            tc.tile_pool(name="tt", bufs=2) as ttpool,
            tc.tile_pool(name="ps", bufs=2, space=bass.MemorySpace.PSUM) as pspool,
            tc.tile_pool(name="po", bufs=6, space=bass.MemorySpace.PSUM) as popool,
            tc.tile_pool(name="eps", bufs=8) as epool,
        ):
            kt_t, qt_t = [], []
            for h in range(H):
                kt = cpool.tile([F_, N], f32, tag=f"kt{h}")
                nc.sync.dma_start(kt[:], kt_d[h])
                kt_t.append(kt)
                qt = cpool.tile([F_, IH], f32, tag=f"qt{h}")
                nc.sync.dma_start(qt[:], qt_d[h])
                qt_t.append(qt)
            va_t = cpool.tile([128, E * H * JB * VA_W], f32, tag="va")
            for e in range(E):
                for h in range(H):
                    for jb in range(JB):
                        s = ((e * H + h) * JB + jb) * VA_W
                        nc.sync.dma_start(va_t[:, s:s + VA_W], va_d[e, h, jb])

            # Phase B: P^T[h, jb] = exp(k_jb @ q^T / 8)   [128 j, 512 i]
            pt_t = {}
            for h in range(H):
                for jb in range(JB):
                    st = pspool.tile([128, IH], f32, tag="st")
                    nc.tensor.matmul(st[:], kt_t[h][:, jb * 128:(jb + 1) * 128],
                                     qt_t[h][:], start=True, stop=True)
                    pt = ptpool.tile([128, IH], f32, tag=f"pt{h}_{jb}")
                    nc.scalar.activation(pt[:], st[:],
                                         mybir.ActivationFunctionType.Exp,
                                         scale=float(1.0 / np.sqrt(F_)))
                    pt_t[h, jb] = pt

            # Phase C: per e: T^T = A^T * P^T, then out = T @ va (accum over jb).
            # h is processed in groups of 2 so tt slots (16 tags x 2 bufs)
            # recycle mid-iteration, letting DVE muls for the next group/e
            # overlap PE accumulation matmuls for the current one.
            for e in range(E):
                at_t = {}
                for jb in range(JB):
                    at = atpool.tile([128, IH], f32, tag=f"at{jb}")
                    nc.sync.dma_start(at[:], at_d[e, jb])
                    at_t[jb] = at
                for hg in range(H // 2):
                    tt_t = {}
                    for jb in range(JB):
                        for h in (2 * hg, 2 * hg + 1):
                            tt = ttpool.tile([128, IH], f32,
                                             tag=f"tt{h % 2}_{jb}")
                            nc.vector.tensor_mul(tt[:], at_t[jb][:],
                                                 pt_t[h, jb][:])
                            tt_t[h, jb] = tt
                    for h in (2 * hg, 2 * hg + 1):
                        col = (e * H + h) * F_
                        for ib in range(IH // 128):
                            po = popool.tile([128, VA_W], f32, tag="po")
                            for jb in range(JB):
                                s = ((e * H + h) * JB + jb) * VA_W
                                nc.tensor.matmul(
                                    po[:],
                                    tt_t[h, jb][:, ib * 128:(ib + 1) * 128],
                                    va_t[:, s:s + VA_W],
                                    start=(jb == 0), stop=(jb == JB - 1))
                            rec = epool.tile([128, 1], f32, tag="rec")
                            nc.vector.reciprocal(rec[:], po[:, F_:F_ + 1])
                            ot = epool.tile([128, F_], f32, tag="ot")
                            nc.scalar.activation(
                                ot[:], po[:, 0:F_],
                                mybir.ActivationFunctionType.Relu,
                                scale=rec[:])
                            nc.sync.dma_start(
                                out_d[ib * 128:(ib + 1) * 128, col:col + F_],
                                ot[:])

    nc.compile()
    return nc


def _prep_core_inputs(b, ih, X, A, kernel_w, biases, aks, akn):
    i0 = ih * IH
    Xb = X[b]                                        # [N, F]
    qt = np.einsum("nf,hfk->hkn", Xb[i0:i0 + IH], aks).astype(np.float32)
    kt = np.einsum("nf,hfk->hkn", Xb, akn).astype(np.float32)
    v = np.einsum("nf,hfk->hnk", Xb, kernel_w)       # [H, N, F_]
    va = np.empty((E, H, JB, 128, VA_W), np.float32)
    for e in range(E):
        for h in range(H):
            vb = v[h] + biases[e, h][None, :]        # [N, F_]
            va[e, h, :, :, :F_] = vb.reshape(JB, 128, F_)
            va[e, h, :, :, F_] = 1.0
    at = np.ascontiguousarray(
        A[b, :, i0:i0 + IH, :].transpose(0, 2, 1)    # [E, N_j, IH]
    ).reshape(E, JB, 128, IH).astype(np.float32)
    return {"qt": qt, "kt": kt, "va": va, "at": at}


def kernel(X, A, kernel, biases, attn_kernel_self, attn_kernel_neighs,
           attn_biases):
    global _compiled
    from concourse import bass_utils

    if _compiled is None:
        _compiled = _build()

    in_maps = [
        _prep_core_inputs(c // 2, c % 2, np.asarray(X), np.asarray(A),
                          np.asarray(kernel), np.asarray(biases),
                          np.asarray(attn_kernel_self),
                          np.asarray(attn_kernel_neighs))
        for c in range(NCORES)
    ]
    res = bass_utils.run_bass_kernel_spmd(_compiled, in_maps,
                                          core_ids=list(range(NCORES)))
    out = np.empty((B, N, E * H * F_), np.float32)
    for c in range(NCORES):
        b, ih = c // 2, c % 2
        out[b, ih * IH:(ih + 1) * IH, :] = res.results[c]["out"]
    return out
